# revision 23
# baseline (speedup 1.0000x reference)
"""Trainium2 Bass kernel for a custom LSTM cell with LayerNorms.

Data-parallel across 8 NeuronCores: batch B=8192 split into 8 shards of 1024
rows; weights replicated and read fp32 straight from HBM (no DRAM cast
roundtrip, no XBAR transposes).

On-chip layout strategy:
  - comb = tanh(LN(concat(x@Wp^T, h))) is built batch-major, LN stats via DVE
    bn_stats (per-partition), then PE-transposed into a feature-major bf16
    tile combT that serves as the *stationary* matmul operand for all four
    gate matmuls.
  - Gate weights stream as fp32 [128,1024] row-chunks, are PE-transposed
    (fp32 transpose matmuls into PSUM, packed 4-per-psum-tile with one
    2KB bank per block) and drained to a double-buffered bf16 W^T slice,
    the *moving* operand. One stationary load (comb block) serves 2 moving
    chunks of 512 out-features.
  - Gate outputs land batch-major: z[batch_part, out_feat]. LN stats are
    per-partition (bn_stats + bn_aggr), applied as per-partition scale/bias;
    the per-feature affine (g, beta) uses Pool tensor ops against
    partition_broadcast tiles.
  - z accumulates over 4 k-chunk-groups directly in bf16 SBUF (psum fp32
    partials, validated < 1e-2 rel err); gate biases are folded into the
    last PSUM chain via a rank-1 ones x bias-row matmul.
  - The whole state update (cell/hidden LN) is batch-major: c loads and
    h/c outputs need no transposes at all.
  - W transposes for matmul-unit u+1 are interleaved into the PSUM-chain
    gaps of unit u to keep the PE stream dense (pstate ramp) and the
    transpose PSUM tile pipelined.
"""

import sys
from contextlib import ExitStack

import numpy as np

sys.path.insert(0, "/opt/trn_rl_repo")

import concourse.bass as bass
import concourse.tile as tile
from concourse import bacc, mybir
from concourse.bass_utils import run_bass_kernel_spmd
from concourse.masks import make_identity

F32 = mybir.dt.float32
BF16 = mybir.dt.bfloat16
AF = mybir.ActivationFunctionType
OP = mybir.AluOpType

B, CIN, H = 8192, 512, 2048
NCORES = 8
BC = B // NCORES            # 1024 batch rows per core
NBT = BC // 128             # 8 batch blocks
H2 = 2 * H                  # 4096
KC = H2 // 128              # 32 contraction chunks for gate matmuls
PC = CIN // 128             # 4 contraction chunks for the input projection
NCG = 2                     # out-feature column groups of 1024
NN = 2                      # 512-wide psum chunks per column group
KCG = 4                     # k chunk groups
K8 = KC // KCG              # 8 k-chunks per group

GATES = ("c2", "i", "f", "o")
GATE_FUNC = {"f": AF.Sigmoid, "i": AF.Sigmoid, "c2": AF.Tanh, "o": AF.Sigmoid}


def build_kernel(nc):
    ins = {}

    def din(name, shape):
        ins[name] = nc.dram_tensor(name, shape, F32, kind="ExternalInput").ap()

    din("x", (BC, 1, CIN))
    din("h", (BC, H))
    din("c", (BC, H))
    din("W_proj", (H, CIN))
    din("b_proj", (H,))
    din("g_ln", (H2,))
    din("b_ln", (H2,))
    din("g_cn", (H,))
    din("b_cn", (H,))
    din("g_hn", (H,))
    din("b_hn", (H,))
    for g in GATES:
        din(f"W_{g}", (H, H2))
        din(f"b_{g}", (H,))
        din(f"g_{g}", (H,))
        din(f"beta_{g}", (H,))

    out_h = nc.dram_tensor("out_h", (BC, H), F32, kind="ExternalOutput").ap()
    out_c = nc.dram_tensor("out_c", (BC, H), F32, kind="ExternalOutput").ap()

    with tile.TileContext(nc) as tc, ExitStack() as ctx:
        build_body(ctx, tc, ins, out_h, out_c)
    nc.compile()
    return nc


def build_body(ctx, tc, ins, out_h, out_c):
    nc = tc.nc

    singles = ctx.enter_context(tc.tile_pool(name="singles", bufs=1))
    small = ctx.enter_context(tc.tile_pool(name="small", bufs=2))
    mm_psum = ctx.enter_context(tc.tile_pool(name="mmpsum", bufs=2, space="PSUM"))
    tp_psum = ctx.enter_context(tc.tile_pool(name="tppsum", bufs=1, space="PSUM"))

    ident_f = singles.tile([128, 128], F32)
    make_identity(nc, ident_f)
    ident_b = singles.tile([128, 128], BF16)
    make_identity(nc, ident_b)
    ones_bf = singles.tile([1, 128], BF16)
    nc.vector.memset(ones_bf, 1.0)
    eps_col = singles.tile([128, 1], F32)
    nc.vector.memset(eps_col, 1e-5)

    # per-feature LN constants for comb, chunk-column layout [p, c] = v[c*128+p]
    def load_cols(name, n_chunks):
        t = singles.tile([128, n_chunks], F32, name=f"cols_{name}")
        nc.sync.dma_start(out=t, in_=ins[name].rearrange("(c p) -> p c", p=128))
        return t

    glncols = load_cols("g_ln", KC)
    blncols = load_cols("b_ln", KC)
    _deferred_brow_proj = True

    brow = {}

    # ---- stats helper (batch-major LN) ------------------------------------
    def ln_stats(src, width, tag):
        """Per-partition (rstd, -mean*rstd) of src [128, width]."""
        nq = width // 512
        st = small.tile([128, nq, 6], F32, tag=f"st_{tag}", name="st")
        for q in range(nq):
            nc.vector.bn_stats(st[:, q, :], src[:, bass.ts(q, 512)])
        mv = small.tile([128, 2], F32, tag=f"mv_{tag}", name="mv")
        nc.vector.bn_aggr(mv, st)
        s = small.tile([128, 1], F32, tag=f"s_{tag}", name="s")
        nc.scalar.activation(out=s, in_=mv[:, 1:2], func=AF.Sqrt, bias=eps_col)
        r = small.tile([128, 1], F32, tag=f"r_{tag}", name="r")
        nc.vector.reciprocal(r, s)
        nm = small.tile([128, 1], F32, tag=f"nm_{tag}", name="nm")
        nc.vector.tensor_mul(nm, mv[:, 0:1], r)
        nc.vector.tensor_scalar_mul(nm, nm, -1.0)
        return r, nm

    # DRAM scratch: prestaged bf16 rows
    dram = tc.alloc_tile_pool(name="dram", bufs=1, space="DRAM")
    ROWV = ["b_proj", "b_c2", "b_i", "b_f", "b_o",
            "g_c2", "beta_c2", "g_i", "beta_i", "g_f", "beta_f",
            "g_o", "beta_o", "g_cn", "b_cn", "g_hn", "b_hn"]
    rows_dram = dram.tile([len(ROWV), H], BF16, name="rows_dram")
    for i, v in enumerate(ROWV):
        rb0 = singles.tile([1, H], BF16, tag="rowb", name="rowb0", bufs=1)
        nc.gpsimd.dma_start(out=rb0, in_=ins[v])
        nc.sync.dma_start(out=rows_dram[i:i + 1, :], in_=rb0)

    def row_load(v):
        """[1, H] bf16 row from the prestaged DRAM copy (sync queue)."""
        rb = singles.tile([1, H], BF16, tag="rowb", name="rowb", bufs=1)
        nc.sync.dma_start(out=rb, in_=rows_dram[ROWV.index(v):ROWV.index(v) + 1, :])
        return rb

    wtp = tc.alloc_tile_pool(name="wtp", bufs=2)
    wstage = tc.alloc_tile_pool(name="wstage", bufs=2)

    def stage_dma_w(name, row0, col0):
        ws = wstage.tile([128, 1024], F32, tag="wsf", name="wsf")
        nc.sync.dma_start(out=ws, in_=ins[name][row0:row0 + 128,
                                                col0:col0 + 1024])
        return ws

    _cast_i = [0]

    def stage_cast(ws):
        wb = wstage.tile([128, 1024], BF16, tag="wsb", name="wsb")
        _cast_i[0] += 1
        if _cast_i[0] % 2 == 0:
            nc.vector.tensor_copy(out=wb, in_=ws)
        else:
            nc.scalar.activation(out=wb, in_=ws, func=AF.Copy)
        return wb

    # ---- Phase 1: x^T, Wp^T, comb (batch-major) + combT (feature-major) ---
    combp = tc.alloc_tile_pool(name="combp", bufs=1)
    combT = combp.tile([128, KC, BC], BF16)  # comb^T [feat, b]

    p1 = tc.alloc_tile_pool(name="p1", bufs=1)
    xT = p1.tile([128, PC, BC], BF16)       # x^T  [cin, b]
    WpT = p1.tile([128, PC, H], BF16)       # Wp^T [cin, ofeat]

    x2d = ins["x"].rearrange("b one k -> (b one) k")
    for bt in range(NBT):
        xs = p1.tile([128, CIN], F32, tag="xstage", name="xs", bufs=2)
        nc.sync.dma_start(out=xs, in_=x2d[bass.ts(bt, 128), :])
        tp = tp_psum.tile([128, 4, 512], F32, tag="tp", name="tp")
        for pc in range(PC):
            nc.tensor.transpose(tp[:, pc, 0:128], xs[:, bass.ts(pc, 128)], ident_f)
        nc.scalar.activation(out=xT[:, 0:PC, bass.ts(bt, 128)],
                             in_=tp[:, :, 0:128], func=AF.Copy)
    for oc in range(H // 128):
        ws = p1.tile([128, CIN], F32, tag="xstage", name="wps", bufs=2)
        nc.sync.dma_start(out=ws, in_=ins["W_proj"][bass.ts(oc, 128), :])
        tp = tp_psum.tile([128, 4, 512], F32, tag="tp", name="tp")
        for pc in range(PC):
            nc.tensor.transpose(tp[:, pc, 0:128], ws[:, bass.ts(pc, 128)], ident_f)
        nc.vector.tensor_copy(out=WpT[:, 0:PC, bass.ts(oc, 128)],
                              in_=tp[:, :, 0:128])

    brow["proj"] = row_load("b_proj")

    tt = {}
    pend = {}
    for bt in range(NBT):
        hst = p1.tile([128, H], F32, tag="hstage", name="hst", bufs=2)
        nc.sync.dma_start(out=hst, in_=ins["h"][bass.ts(bt, 128), :])
        # xp = x @ Wp^T + b_proj  (batch-major, 4 psum chains of 512)
        mm = [mm_psum.tile([128, NN, 512], F32, tag="mm", name="mm")
              for _ in range(2)]
        for pc in range(PC):
            lhs = xT[:, pc, bass.ts(bt, 128)]
            for j in range(4):
                nc.tensor.matmul(mm[j // 2][:, j % 2, :], lhs,
                                 WpT[:, pc, bass.ts(j, 512)],
                                 start=(pc == 0), stop=False)
        for j in range(4):
            nc.tensor.matmul(mm[j // 2][:, j % 2, :], ones_bf,
                             brow["proj"][:, bass.ts(j, 512)],
                             start=False, stop=True)
        craw = p1.tile([128, H2], BF16, tag="craw", name="craw", bufs=2)
        for j2 in range(2):
            nc.vector.tensor_copy(out=craw[:, bass.ts(j2, 1024)], in_=mm[j2])
        nc.scalar.activation(out=craw[:, bass.ts(1, H)], in_=hst, func=AF.Copy)
        r, nm = ln_stats(craw, H2, "c")
        t = p1.tile([128, H2], BF16, tag=f"t{bt % 4}", name="t", bufs=1)
        nc.vector.tensor_scalar(out=t, in0=craw, scalar1=r, scalar2=nm,
                                op0=OP.mult, op1=OP.add)
        tt[bt] = t
        # prestage gate c2 / unit 0's weight chunks during phase 1
        pend[("f", 0, bt)] = stage_dma_w("W_c2", bt * 128, 0)
        if bt >= 1:
            pend[("b", 0, bt - 1)] = stage_cast(pend.pop(("f", 0, bt - 1)))
        if bt % 4 == 3:
            half = bt // 4
            for kc in range(KC):
                tpb = tp_psum.tile([128, 4, 1024], BF16, tag="tp", name="tpb")
                for j in range(4):
                    nc.tensor.transpose(tpb[:, j, 0:128],
                                        tt[half * 4 + j][:, bass.ts(kc, 128)],
                                        ident_b)
                dst = combT[:, kc, bass.ts(half, 512)]
                nc.scalar.activation(
                    out=dst.rearrange("p (j b) -> p j b", j=4),
                    in_=tpb[:, :, 0:128], func=AF.Tanh,
                    scale=glncols[:, kc:kc + 1], bias=blncols[:, kc:kc + 1])
    p1.release()

    # ---- Phase 2: gates ---------------------------------------------------
    zpool = tc.alloc_tile_pool(name="zpool", bufs=1)
    gbcp = tc.alloc_tile_pool(name="gbcp", bufs=1)
    cellp = tc.alloc_tile_pool(name="cellp", bufs=2)


    zA = {bb: zpool.tile([128, H], BF16, tag=f"zA{bb}", name=f"zA{bb}")
          for bb in range(NBT)}
    zB = {}

    units = [(g, ncg, kcg) for g in GATES for ncg in range(NCG)
             for kcg in range(KCG)]

    wt_tiles = {}

    def get_wt(unit):
        if unit not in wt_tiles:
            wt_tiles[unit] = wtp.tile([128, K8, 1024], BF16, tag="wt", name="wt")
        return wt_tiles[unit]

    def stage_dma(unit, oc):
        g, ncg, kcg = unit
        return stage_dma_w(f"W_{g}", ncg * 1024 + oc * 128, kcg * 1024)

    def transpose_unit_oc(unit, oc, wb):
        """PE-transpose one bf16 staged row-chunk into unit's W^T slice."""
        wt = get_wt(unit)
        for q in range(2):
            tp = tp_psum.tile([128, 4, 1024], BF16, tag="tp", name="wtps")
            for j in range(4):
                k8 = q * 4 + j
                nc.tensor.transpose(tp[:, j, 0:128],
                                    wb[:, bass.ts(k8, 128)], ident_b)
            dst = wt[:, q * 4:(q + 1) * 4, bass.ts(oc, 128)]
            if (oc + q) % 2 == 0:
                nc.scalar.activation(out=dst, in_=tp[:, :, 0:128], func=AF.Copy)
            else:
                nc.vector.tensor_copy(out=dst, in_=tp[:, :, 0:128])

    def build_affine(gname, tag_g, tag_b):
        """Partition-broadcast bf16 [128, H] tiles of g_<name>, beta/b_<name>."""
        out = []
        bname = f"beta_{gname}" if gname in GATES else f"b_{gname}"
        for tag, src in ((tag_g, f"g_{gname}"), (tag_b, bname)):
            rb = row_load(src)
            bc = gbcp.tile([128, H], BF16, tag=tag, name="bc")
            nc.gpsimd.partition_broadcast(bc, rb)
            out.append(bc)
        return out

    # prologue: finish cast + transpose unit 0 (staged during phase 1)
    pend[("b", 0, 7)] = stage_cast(pend.pop(("f", 0, 7)))
    for oc in range(8):
        transpose_unit_oc(units[0], oc, pend.pop(("b", 0, oc)))

    gbc, betabc = {}, {}
    cst_tiles = {}

    def state_out(src_bf, gb, bb_, dst_dram, tag, aff_eng):
        """LN-affine src (per-partition stats) to fp32 halves, DMA out.
        Returns the two half tiles for further use."""
        r, nm = ln_stats(src_bf, H, tag)
        halves = []
        for hh in range(2):
            cf = cellp.tile([128, H // 2], F32, tag="cell", name="cf",
                            bufs=1)
            nc.vector.tensor_scalar(out=cf, in0=src_bf[:, bass.ts(hh, H // 2)],
                                    scalar1=r, scalar2=nm,
                                    op0=OP.mult, op1=OP.add)
            aff_eng.tensor_mul(cf, cf, gb[0][:, bass.ts(hh, H // 2)])
            aff_eng.tensor_add(cf, cf, gb[1][:, bass.ts(hh, H // 2)])
            halves.append(cf)
        return halves

    def apply_gate(g, bb):
        """LN affine + nonlinearity on Z, then state fusion for this bb."""
        Z = zA[bb] if g == "c2" else zB[bb]
        eng = nc.gpsimd if g in ("c2", "i") else nc.vector
        r, nm = ln_stats(Z, H, "z")
        nc.vector.tensor_scalar(out=Z, in0=Z, scalar1=r, scalar2=nm,
                                op0=OP.mult, op1=OP.add)
        eng.tensor_mul(Z, Z, gbc[g])
        eng.tensor_add(Z, Z, betabc[g])
        nc.scalar.activation(out=Z, in_=Z, func=GATE_FUNC[g])
        if g == "i":
            nc.gpsimd.tensor_mul(zA[bb], Z, zA[bb])          # v = i * cc
        elif g == "f":
            cs = cst_tiles.pop(bb)
            nc.vector.tensor_mul(Z, Z, cs)                   # f * c
            nc.vector.tensor_add(zA[bb], Z, zA[bb])          # cp = f*c + v
            # cell LN -> out_c; tc = tanh(cell) into zA
            halves = state_out(zA[bb], (gbc["cn"], betabc["cn"]), bb, out_c,
                               "cell", nc.vector)
            for hh, cf in enumerate(halves):
                nc.scalar.dma_start(
                    out=out_c[bass.ts(bb, 128), bass.ts(hh, H // 2)], in_=cf)
                nc.scalar.activation(out=zA[bb][:, bass.ts(hh, H // 2)],
                                     in_=cf, func=AF.Tanh)
        elif g == "o":
            nc.vector.tensor_mul(zA[bb], Z, zA[bb])          # hp = o * tc
            halves = state_out(zA[bb], (gbc["hn"], betabc["hn"]), bb, out_h,
                               "hid", nc.gpsimd)
            for hh, cf in enumerate(halves):
                nc.scalar.activation(out=cf, in_=cf, func=AF.Tanh)
                nc.scalar.dma_start(
                    out=out_h[bass.ts(bb, 128), bass.ts(hh, H // 2)], in_=cf)

    for ui, unit in enumerate(units):
        g, ncg, kcg = unit
        nxt = units[ui + 1] if ui + 1 < len(units) else None
        if ncg == 0 and kcg == 0:
            if g != "c2":
                zB = {bb: zpool.tile([128, H], BF16, tag=f"zB{bb}",
                                     name=f"zB{bb}")
                      for bb in range(NBT)}
            gbc[g], betabc[g] = build_affine(g, "gbc", "betabc")
            if g == "f":
                gbc["cn"], betabc["cn"] = build_affine("cn", "gaff", "baff")
            if g == "o":
                gbc["hn"], betabc["hn"] = build_affine("hn", "gaff", "baff")
            brow[g] = row_load(f"b_{g}")
        if nxt is not None:
            pend[("f", ui + 1, 0)] = stage_dma(nxt, 0)
            pend[("f", ui + 1, 1)] = stage_dma(nxt, 1)
            pend[("b", ui + 1, 0)] = stage_cast(pend.pop(("f", ui + 1, 0)))
        wt = get_wt(unit)
        Zs = zA if g == "c2" else zB
        last = (ncg == NCG - 1 and kcg == KCG - 1)
        for bb in range(NBT):
            mm = mm_psum.tile([128, NN, 512], F32, tag="mm", name="gmm")
            for k8 in range(K8):
                kc = kcg * K8 + k8
                lhs = combT[:, kc, bass.ts(bb, 128)]
                for j in range(NN):
                    nc.tensor.matmul(mm[:, j, :], lhs, wt[:, k8, bass.ts(j, 512)],
                                     start=(k8 == 0),
                                     stop=(k8 == K8 - 1 and kcg != KCG - 1))
            if kcg == KCG - 1:
                for j in range(NN):
                    nc.tensor.matmul(mm[:, j, :], ones_bf,
                                     brow[g][:, ncg * 1024 + j * 512:
                                             ncg * 1024 + (j + 1) * 512],
                                     start=False, stop=True)
            dst = Zs[bb][:, bass.ts(ncg, 1024)].rearrange("p (j n) -> p j n",
                                                          j=NN)
            if kcg == 0:
                nc.scalar.activation(out=dst, in_=mm, func=AF.Copy)
            else:
                nc.vector.tensor_add(dst, mm, dst)
            # prefetch c (sync queue, fp32) during the f gate for the fusion
            if g == "f" and ncg == 0 and kcg == 3:
                cs = cellp.tile([128, H], F32, tag="cstage", name="cs", bufs=1)
                nc.sync.dma_start(out=cs, in_=ins["c"][bass.ts(bb, 128), :])
                cst_tiles[bb] = cs
            # interleave next unit's W staging pipeline into the gaps:
            # transpose oc=bb (cast at gap bb-1, loaded at gap bb-2)
            if nxt is not None:
                transpose_unit_oc(nxt, bb, pend.pop(("b", ui + 1, bb)))
                if bb + 1 < 8:
                    pend[("b", ui + 1, bb + 1)] = stage_cast(
                        pend.pop(("f", ui + 1, bb + 1)))
                if bb + 2 < 8:
                    pend[("f", ui + 1, bb + 2)] = stage_dma(nxt, bb + 2)
            if last:
                apply_gate(g, bb)
        wt_tiles.pop(unit)

    cellp.release()
    gbcp.release()
    zpool.release()
    combp.release()
    wstage.release()
    wtp.release()
    dram.release()


_NC_CACHE = {}


def _get_nc():
    if "nc" not in _NC_CACHE:
        nc = bacc.Bacc(
            "TRN2",
            target_bir_lowering=False,
            debug=False,
            enable_asserts=False,
            num_devices=NCORES,
        )
        _NC_CACHE["nc"] = build_kernel(nc)
    return _NC_CACHE["nc"]


def run(inputs, **kw):
    nc = _get_nc()
    full = {k: np.ascontiguousarray(np.asarray(v, dtype=np.float32))
            for k, v in inputs.items()}
    in_maps = []
    for i in range(NCORES):
        s = slice(i * BC, (i + 1) * BC)
        m = {k: (np.ascontiguousarray(v[s]) if k in ("x", "h", "c") else v)
             for k, v in full.items()}
        in_maps.append(m)
    res = run_bass_kernel_spmd(nc, in_maps, core_ids=list(range(NCORES)), **kw)
    nh = np.concatenate([r["out_h"] for r in res.results], axis=0)
    ncl = np.concatenate([r["out_c"] for r in res.results], axis=0)
    return np.stack([nh, ncl]).astype(np.float32), res


def kernel(**inputs) -> np.ndarray:
    out, _ = run(inputs)
    return out
